# revision 1
# baseline (speedup 1.0000x reference)
"""LongAxisSelfAttention Trainium2 kernel (8-core SPMD, Bass/Tile).

Problem: B=2, S=4096, H=768, 12 heads x 64: heads 0-5 full attention,
heads 6-11 4-way strided ("axis") attention.

Sharding (uniform SPMD program, data-parameterized per core):
  core c: batch b=c//4, ci=c%4.
    full heads  F = [0,1,2] if ci<2 else [3,4,5], q-half qh=ci%2
    axis heads  A = [6,7,8] if ci<2 else [9,10,11], groups (0,1) or (2,3)

Math layout: everything in transposed space. scoresT[k,q] = KT.T @ QT
(contraction over head dim on partitions), exp with the 1/sqrt(64)
scale folded in, then ctxT[d,q] (+ sums row via ones-augmented V)
= [V|1].T @ expsT with PSUM k-accumulation, PE-transpose back to
[q,d], reciprocal normalize, DMA out.

Perf history (HW-measured): f32r baseline 674us -> bf16 408us ->
pipeline/granularity 374us -> uniform-row K + runs-of-4 329us ->
DMA-layout + ordering 327us.  Rel err 1.05e-2 (gate 2e-2).

Perf notes (measured on trn2):
- bf16 matmul operands: fp32r draws enough power that the PE clock is
  firmware-throttled to K=4/8 (1.2 GHz) sustained; bf16 runs at 2.4.
  fp8 (w/ or w/o DoubleRow) is both slower for this shape and fails
  the 2e-2 accuracy gate.
- K is stored per-head in half-zeroed 128-row tiles so every matmul
  has a 128-row stationary operand: alternating 64/128-row weights
  costs a large PE reconfig bubble (probe: 553 vs 219 ns/MM).
- scores/ctx matmuls issue in runs of 4 (4 scores, then 4 ctx
  accumulates) - grouping same-kind matmuls avoids single<->accum
  group-switch overhead.
- exp split between ScalarE (exact) and VectorE (Schraudolph: bf16
  bits = round(23.083*s + 16250.5) as int16, ~+-3% on weights;
  HW-validated rel err 1.05e-2 < 2e-2).
- single Tile scope, shared PSUM tags, emission order A, C-full, B,
  C-axis so attention matmuls fill projection stalls and vice versa.
"""

import numpy as np

B, S, H = 2, 4096, 768
NH, D, SEG = 12, 64, 6
P = 128
KT_H = H // P            # 6 hidden k-tiles
QHALF = S // 2           # 2048
AXLEN = S // 2           # per-core axis length (2 groups x 1024)
GLEN = S // 4            # 1024

# Schraudolph fast-exp constants: bf16 bits of exp(0.125*s) ~=
# int16(round(128*(0.125*s*log2(e) + 127 - 0.0429)))
SCHRAUD_A = 128.0 * 0.125 * 1.4426950408889634
SCHRAUD_B = 128.0 * (127.0 - 0.0429)

_CACHE = {}


def _build_nc():
    import concourse.bass as bass
    import concourse.mybir as mybir
    import concourse.tile as tile
    from concourse import bacc
    from contextlib import ExitStack

    F32 = mybir.dt.float32
    I16 = mybir.dt.int16
    I32 = mybir.dt.int32
    MMDT = mybir.dt.bfloat16      # matmul operand dtype
    AF = mybir.ActivationFunctionType
    MUL = mybir.AluOpType.mult
    ADD = mybir.AluOpType.add

    nc = bacc.Bacc(None, target_bir_lowering=False)

    # ---- DRAM I/O ----
    # all host-marshalled into per-partition-contiguous layouts: the
    # natural "(o p) -> p o" rearrange DMA generates 768B-line descriptors
    # (~768 per transfer); these layouts need only 128.
    hT = nc.dram_tensor("hT", [P, (S // 1024) * KT_H * 1024], MMDT,
                        kind="ExternalInput")
    hT_ax = nc.dram_tensor("hT_ax", [P, (AXLEN // 1024) * KT_H * 1024], MMDT,
                           kind="ExternalInput")
    w_kv = nc.dram_tensor("w_kv", [P, KT_H * 384], MMDT, kind="ExternalInput")
    w_q = nc.dram_tensor("w_q", [P, KT_H * 256], MMDT, kind="ExternalInput")
    w_ax = nc.dram_tensor("w_ax", [P, KT_H * 640], MMDT, kind="ExternalInput")
    b_all = nc.dram_tensor("b_all", [1280], F32, kind="ExternalInput")
    ident_f = nc.dram_tensor("ident_f", [P, P], F32, kind="ExternalInput")
    ident_r = nc.dram_tensor("ident_r", [P, P], MMDT, kind="ExternalInput")
    out_full = nc.dram_tensor("out_full", [P, 3, QHALF // P, 64], F32,
                              kind="ExternalOutput")
    out_ax = nc.dram_tensor("out_ax", [P, 3, AXLEN // P, 64], F32,
                            kind="ExternalOutput")

    # K: per-unit half-zeroed [P, S'] tiles (uniform 128-row stationary).
    # unit u's real rows sit at base KBASE[u]; the other 64 rows are 0.
    KBASE = [0, 64, 0]
    QM_SLOT = [0, 0, 1]          # unit -> q m-tile (full-P rhs view)
    # V^T slots (base_partition, m_tile) in vT_sb / vaxT_sb
    VT_SLOT = [(64, 0), (0, 1), (64, 1)]
    AXVT_SLOT = [(64, 0), (64, 1), (0, 1)]

    with tile.TileContext(nc) as tc, ExitStack() as top:
        constp = top.enter_context(tc.tile_pool(name="constp", bufs=1))
        persist = top.enter_context(tc.tile_pool(name="persist", bufs=1))
        wp = top.enter_context(tc.tile_pool(name="wp", bufs=1))
        hT_pool = top.enter_context(tc.tile_pool(name="hT_pool", bufs=2))
        hax_pool = top.enter_context(tc.tile_pool(name="hax_pool", bufs=2))
        # PSUM: "mm" [P,512]f32 (1 bank) x5 slots shared by proj+scores;
        # "ctx" [65,512] x1; "tr" 1 bank x2.  5+1+2 = 8 banks.
        pps = top.enter_context(tc.tile_pool(name="ps_mm", bufs=5, space="PSUM"))
        ctxp = top.enter_context(tc.tile_pool(name="ps_ctx", bufs=1, space="PSUM"))
        trp = top.enter_context(tc.tile_pool(name="ps_tr", bufs=2, space="PSUM"))
        exps = top.enter_context(tc.tile_pool(name="exps", bufs=8))
        epi = top.enter_context(tc.tile_pool(name="epi", bufs=3))

        id_f = constp.tile([P, P], F32)
        nc.sync.dma_start(id_f[:], ident_f[:])
        id_r = constp.tile([P, P], MMDT)
        nc.sync.dma_start(id_r[:], ident_r[:])
        bias_sb = constp.tile([P, 10], F32)
        nc.sync.dma_start(bias_sb[:], b_all.rearrange("(m p) -> p m", p=P))

        # persistent projection outputs
        kT3 = persist.tile([P, 3, S], MMDT)          # 24KB/part
        vT_sb = persist.tile([P, 2, S], MMDT)        # 16KB
        qT_sb = persist.tile([P, 2, QHALF], MMDT)    # 8KB
        kaT3 = persist.tile([P, 3, AXLEN], MMDT)     # 12KB
        vaxT_sb = persist.tile([P, 2, AXLEN], MMDT)  # 8KB
        qaT_sb = persist.tile([P, 2, AXLEN], MMDT)   # 8KB
        # inner stride 80 (160B): xbar DMA-transpose needs 32B-aligned dsts
        v_sb = persist.tile([P, S // P, 3, 80], MMDT)     # 15KB
        vax_sb = persist.tile([P, AXLEN // P, 3, 80], MMDT)  # 7.5KB

        # zero halves of the K tiles (and the never-written qaT half):
        # zero x garbage would be NaN-risky, zero x real-Q contributes 0.
        for u in range(3):
            zb = 64 - KBASE[u]
            nc.vector.memset(kT3[zb:zb + 64, u, :].bitcast(I32), 0)
            nc.vector.memset(kaT3[zb:zb + 64, u, :].bitcast(I32), 0)
        nc.vector.memset(qaT_sb[64:P, 1, :].bitcast(I32), 0)

        # wkv col-block 0 feeds the very first projection group; the host
        # lays it out as a contiguous leading section so it can land as
        # its own (descriptor-cheap) DMA before the rest of wkv.
        wkv_sb = wp.tile([P, KT_H, 384], MMDT)
        nc.sync.dma_start(
            wkv_sb[:, :, 0:P],
            w_kv[:, 0:KT_H * P].rearrange("p (o m) -> p o m", o=KT_H))
        nc.sync.dma_start(
            wkv_sb[:, :, P:384],
            w_kv[:, KT_H * P:].rearrange("p (o m) -> p o m", o=KT_H))
        wq_sb = wp.tile([P, KT_H, 256], MMDT)
        wax_sb = wp.tile([P, KT_H, 640], MMDT)

        hview = hT.rearrange("p (c o s) -> p c o s", c=S // 1024, o=KT_H)
        haxview = hT_ax.rearrange("p (c o s) -> p c o s", c=AXLEN // 1024, o=KT_H)
        CH = 1024

        def proj_ps(wsb, wcol0, rhs, rsl):
            """One [P,512] PSUM tile = (w[:, wcol0:+128].T @ rhs[:, rsl])."""
            ps = pps.tile([P, 512], F32, name="proj", tag="mmps")
            for k in range(KT_H):
                nc.tensor.matmul(
                    ps[:], wsb[:, k, wcol0:wcol0 + P], rhs[:, k, rsl],
                    start=(k == 0), stop=(k == KT_H - 1),
                )
            return ps

        def epilogue(ps, dsts):
            """dsts: list of (dst_ap, bias_m, part_lo, part_hi)."""
            for dst, bias_m, lo, hi in dsts:
                nc.scalar.activation(
                    dst, ps[lo:hi], AF.Identity,
                    bias=bias_sb[lo:hi, bias_m:bias_m + 1],
                )

        # ---------------- phase A: full-head projections ----------------
        # w_kv cols: [K0 K1 | K2 V0 | V1 V2], w_q cols: [Q0 Q1 | Q2 z]
        for ch in range(S // CH):
            hch = hT_pool.tile([P, KT_H, CH], MMDT, name="hch", tag="hch")
            if ch == 0:
                # split the first chunk's DMA so the first projection can
                # start as soon as the first 512 columns land
                for half in range(2):
                    hsl = slice(half * 512, (half + 1) * 512)
                    nc.sync.dma_start(hch[:, :, hsl], hview[:, 0, :, hsl])
                nc.sync.dma_start(
                    wq_sb[:], w_q.rearrange("p (o m) -> p o m", o=KT_H))
                nc.sync.dma_start(
                    wax_sb[:], w_ax.rearrange("p (o m) -> p o m", o=KT_H))
            else:
                nc.sync.dma_start(hch[:], hview[:, ch])
            for n in range(CH // 512):
                rsl = slice(n * 512, (n + 1) * 512)
                dsl = slice(ch * CH + n * 512, ch * CH + (n + 1) * 512)
                ps = proj_ps(wkv_sb, 0, hch, rsl)
                epilogue(ps, [(kT3[0:64, 0, dsl], 0, 0, 64),
                              (kT3[64:P, 1, dsl], 0, 64, P)])
                ps = proj_ps(wkv_sb, P, hch, rsl)
                epilogue(ps, [(kT3[0:64, 2, dsl], 1, 0, 64),
                              (vT_sb[64:P, 0, dsl], 1, 64, P)])
                ps = proj_ps(wkv_sb, 2 * P, hch, rsl)
                epilogue(ps, [(vT_sb[:, 1, dsl], 2, 0, P)])
                if ch < QHALF // CH:
                    ps = proj_ps(wq_sb, 0, hch, rsl)
                    epilogue(ps, [(qT_sb[:, 0, dsl], 3, 0, P)])
                    ps = proj_ps(wq_sb, P, hch, rsl)
                    epilogue(ps, [(qT_sb[:, 1, dsl], 4, 0, P)])
            # V transposes for this chunk's 8 P-blocks (PE transpose: the
            # xbar DMA-transpose alternative serializes DMA queues on mode
            # transitions and measured slower end-to-end)
            for u in range(3):
                base, mt = VT_SLOT[u]
                tp = trp.tile([P, 8, 64], MMDT, name="vtr", tag="trps")
                for j in range(8):
                    cc = ch * 8 + j
                    nc.tensor.transpose(
                        tp[:, j, :],
                        vT_sb[base:base + 64, mt, cc * P:(cc + 1) * P],
                        id_r[base:base + 64, base:base + 64],
                    )
                nc.vector.tensor_copy(v_sb[:, ch * 8:ch * 8 + 8, u, 0:64], tp[:])
        # ones columns: x*0 + 1 (source must be real data, not garbage)
        nc.vector.tensor_scalar(
            v_sb[:, :, :, 64:65],
            vT_sb[:, 1, 0:96].rearrange("p (c u) -> p c u", c=S // P)[:, :, :, None],
            0.0, 1.0, op0=MUL, op1=ADD,
        )

        # ---------------- attention unit ----------------
        def attn_unit2(kT, qT, vsb, u, qm, k0, q0, nk, nq, outv, oc0):
            for qs in range(nq // 512):
                q_sl = slice(q0 + qs * 512, q0 + (qs + 1) * 512)
                ctx_ps = ctxp.tile([65, 512], F32, name="ctxps", tag="ctxps")
                nkb = nk // P
                for k4 in range(nkb // 4):
                    exs = []
                    # run of 4 score matmuls (+ their exps)
                    for i in range(4):
                        kb = k4 * 4 + i
                        ksl = slice(k0 + kb * P, k0 + (kb + 1) * P)
                        sc = pps.tile([P, 512], F32, name="scps", tag="mmps")
                        ex = exps.tile([P, 512], MMDT, name="exsb", tag="exsb")
                        exs.append(ex)
                        nc.tensor.matmul(sc[:], kT[:, u, ksl], qT[:, qm, q_sl],
                                         start=True, stop=True)
                        if kb % 2 == 1:
                            # Schraudolph fast-exp on VectorE (bf16 via i16)
                            nc.vector.tensor_scalar(
                                ex[:].bitcast(I16), sc[:],
                                SCHRAUD_A, SCHRAUD_B, op0=MUL, op1=ADD,
                            )
                        else:
                            nc.scalar.activation(ex[:], sc[:], AF.Exp,
                                                 scale=0.125)
                    # run of 4 ctx accumulates
                    for i in range(4):
                        kb = k4 * 4 + i
                        kc = (k0 + kb * P) // P
                        nc.tensor.matmul(
                            ctx_ps[:], vsb[:, kc, u, 0:65], exs[i][:],
                            start=(kb == 0),
                            stop=(kb == nkb - 1),
                        )
                # epilogue
                ctxT_sb = epi.tile([65, 512], F32, name="ctxTsb", tag="ctxTsb")
                nc.vector.tensor_copy(ctxT_sb[:], ctx_ps[:])
                tp = trp.tile([P, 4, 65], F32, name="tpps", tag="trps")
                for j in range(4):
                    nc.tensor.transpose(
                        tp[:, j, :], ctxT_sb[:, j * P:(j + 1) * P],
                        id_f[0:65, 0:65],
                    )
                recip = epi.tile([P, 4], F32, name="recip", tag="recip")
                nc.vector.reciprocal(recip[:], tp[:, :, 64])
                outsb = epi.tile([P, 4, 64], F32, name="outsb", tag="outsb")
                nc.vector.tensor_tensor(
                    outsb[:], tp[:, :, 0:64],
                    recip[:, :, None].to_broadcast([P, 4, 64]), op=MUL,
                )
                c0 = (oc0 + qs * 512) // P
                nc.sync.dma_start(outv[:, u, c0:c0 + 4, :], outsb[:])

        # ---------------- C-full (emitted before B: fills B's stalls) ----
        for u in range(3):
            attn_unit2(kT3, qT_sb, v_sb, u, QM_SLOT[u],
                       0, 0, S, QHALF, out_full, 0)

        # ---------------- phase B: axis projections ----------------
        # w_ax cols: [kA0 kA1 | qA0 qA1 | kA2 vA0 | qA2 vA1 | vA2 z]
        for ch in range(AXLEN // CH):
            hch = hax_pool.tile([P, KT_H, CH], MMDT, name="hch2", tag="hch2")
            nc.sync.dma_start(hch[:], haxview[:, ch])
            for n in range(CH // 512):
                rsl = slice(n * 512, (n + 1) * 512)
                dsl = slice(ch * CH + n * 512, ch * CH + (n + 1) * 512)
                ps = proj_ps(wax_sb, 0, hch, rsl)
                epilogue(ps, [(kaT3[0:64, 0, dsl], 5, 0, 64),
                              (kaT3[64:P, 1, dsl], 5, 64, P)])
                ps = proj_ps(wax_sb, P, hch, rsl)
                epilogue(ps, [(qaT_sb[:, 0, dsl], 6, 0, P)])
                ps = proj_ps(wax_sb, 2 * P, hch, rsl)
                epilogue(ps, [(kaT3[0:64, 2, dsl], 7, 0, 64),
                              (vaxT_sb[64:P, 0, dsl], 7, 64, P)])
                ps = proj_ps(wax_sb, 3 * P, hch, rsl)
                epilogue(ps, [(qaT_sb[0:64, 1, dsl], 8, 0, 64),
                              (vaxT_sb[64:P, 1, dsl], 8, 64, P)])
                ps = proj_ps(wax_sb, 4 * P, hch, rsl)
                epilogue(ps, [(vaxT_sb[0:64, 1, dsl], 9, 0, 64)])
            for u in range(3):
                base, mt = AXVT_SLOT[u]
                tp = trp.tile([P, 8, 64], MMDT, name="vtr2", tag="trps")
                for j in range(8):
                    cc = ch * 8 + j
                    nc.tensor.transpose(
                        tp[:, j, :],
                        vaxT_sb[base:base + 64, mt, cc * P:(cc + 1) * P],
                        id_r[base:base + 64, base:base + 64],
                    )
                nc.vector.tensor_copy(
                    vax_sb[:, ch * 8:ch * 8 + 8, u, 0:64], tp[:])
        nc.vector.tensor_scalar(
            vax_sb[:, :, :, 64:65],
            vaxT_sb[:, 1, 0:48].rearrange("p (c u) -> p c u", c=AXLEN // P)[:, :, :, None],
            0.0, 1.0, op0=MUL, op1=ADD,
        )

        # ---------------- C-axis ----------------
        for u in range(3):
            for g in range(2):
                attn_unit2(kaT3, qaT_sb, vax_sb, u, QM_SLOT[u],
                           g * GLEN, g * GLEN, GLEN, GLEN,
                           out_ax, g * GLEN)

    nc.finalize()
    return nc


def _get_nc():
    if "nc" not in _CACHE:
        _CACHE["nc"] = _build_nc()
    return _CACHE["nc"]


def _prep_inputs(hidden_states, Wq, bq, Wk, bk, Wv, bv):
    """Build the 8 per-core input maps (host-side marshalling)."""
    import ml_dtypes
    BF16 = ml_dtypes.bfloat16
    hs = np.ascontiguousarray(hidden_states, dtype=np.float32)
    eye = np.eye(P, dtype=np.float32)
    in_maps = []
    for c in range(8):
        b, ci = divmod(c, 4)
        F0 = 0 if ci < 2 else 3          # first full head
        A0 = 6 if ci < 2 else 9          # first axis head
        qh = ci % 2
        ga, gb = (0, 1) if ci % 2 == 0 else (2, 3)

        hb = hs[b]                        # [S, H]
        # permuted: our q-half first
        hperm = np.concatenate([hb[qh * QHALF:(qh + 1) * QHALF],
                                hb[(1 - qh) * QHALF:(2 - qh) * QHALF]], axis=0)
        hax = np.concatenate([hb[ga::4], hb[gb::4]], axis=0)

        def chunked_T(x):
            # [S', H] -> per-partition-contiguous [128, c*6*1024]:
            # dst[p, c, o, s'] = x.T[o*128+p, c*1024+s']
            sl = x.shape[0]
            xT = x.T.reshape(KT_H, P, sl // 1024, 1024)
            return np.ascontiguousarray(
                xT.transpose(1, 2, 0, 3).reshape(P, -1))

        hT = chunked_T(hperm)
        hT_ax = chunked_T(hax)

        def rows(W, h0, n=3):
            return [W[64 * (h0 + i):64 * (h0 + i) + 64] for i in range(n)]

        kf = rows(Wk, F0); vf = rows(Wv, F0); qf = rows(Wq, F0)
        ka = rows(Wk, A0); va = rows(Wv, A0); qa = rows(Wq, A0)
        z64 = np.zeros((64, H), np.float32)

        def wprep(wmat):
            # [H, M] -> [128, 6*M]: dst[p, o*M+m] = wmat[o*128+p, m]
            M = wmat.shape[1]
            return np.ascontiguousarray(
                wmat.reshape(KT_H, P, M).transpose(1, 0, 2).reshape(P, -1))

        # w_kv: col-block 0 (K0|K1) as a contiguous leading section
        wkv_mat = np.concatenate(
            [kf[0], kf[1], kf[2], vf[0], vf[1], vf[2]]).T  # [H, 384]
        w_kv = np.concatenate(
            [wprep(wkv_mat[:, 0:P]), wprep(wkv_mat[:, P:384])], axis=1)
        w_q = wprep(np.concatenate([qf[0], qf[1], qf[2], z64]).T)
        w_ax = wprep(np.concatenate(
            [ka[0], ka[1], qa[0], qa[1],
             ka[2], va[0], qa[2], va[1], va[2], z64]).T)

        def brows(bvec, h0, i):
            return bvec[64 * (h0 + i):64 * (h0 + i) + 64]

        z64b = np.zeros(64, np.float32)
        b_kv = np.concatenate([brows(bk, F0, 0), brows(bk, F0, 1),
                               brows(bk, F0, 2), brows(bv, F0, 0),
                               brows(bv, F0, 1), brows(bv, F0, 2)])
        b_q = np.concatenate([brows(bq, F0, 0), brows(bq, F0, 1),
                              brows(bq, F0, 2), z64b])
        b_ax = np.concatenate([brows(bk, A0, 0), brows(bk, A0, 1),
                               brows(bq, A0, 0), brows(bq, A0, 1),
                               brows(bk, A0, 2), brows(bv, A0, 0),
                               brows(bq, A0, 2), brows(bv, A0, 1),
                               brows(bv, A0, 2), z64b])
        b_all = np.concatenate([b_kv, b_q, b_ax]).astype(np.float32)

        in_maps.append({
            "hT": hT.astype(BF16), "hT_ax": hT_ax.astype(BF16),
            "w_kv": w_kv.astype(BF16), "w_q": w_q.astype(BF16),
            "w_ax": w_ax.astype(BF16),
            "b_all": b_all, "ident_f": eye, "ident_r": eye.astype(BF16),
        })
    return in_maps


def _assemble(results):
    out = np.empty((B, S, H), np.float32)
    for c in range(8):
        b, ci = divmod(c, 4)
        F0 = 0 if ci < 2 else 3
        A0 = 6 if ci < 2 else 9
        qh = ci % 2
        ga, gb = (0, 1) if ci % 2 == 0 else (2, 3)
        r = results[c]
        # [128, 3, c, 64] -> [c*128, 192]
        of = r["out_full"].transpose(2, 0, 1, 3).reshape(QHALF, 192)
        oa = r["out_ax"].transpose(2, 0, 1, 3).reshape(AXLEN, 192)
        out[b, qh * QHALF:(qh + 1) * QHALF, 64 * F0:64 * F0 + 192] = of
        out[b, ga::4, 64 * A0:64 * A0 + 192] = oa[:GLEN]
        out[b, gb::4, 64 * A0:64 * A0 + 192] = oa[GLEN:]
    return out


def run(inputs, trace=False):
    from concourse.bass_utils import run_bass_kernel_spmd
    nc = _get_nc()
    in_maps = _prep_inputs(**inputs)
    res = run_bass_kernel_spmd(nc, in_maps, core_ids=list(range(8)), trace=trace)
    return _assemble(res.results), res


def kernel(**inputs):
    out, _ = run(inputs, trace=False)
    return out

